# revision 30
# baseline (speedup 1.0000x reference)
"""Dice coefficient metric kernel for TRN2 (8 NeuronCores, SPMD batch-parallel).

Reference computation (all fp32):
    inter[b,c] = sum_hw prd*tgt
    union[b,c] = sum_hw prd + sum_hw tgt + EPS
    dice[b,c]  = (2*inter + EPS) / union
    out[c]     = mean_b dice[b,c]

Sharding: batch dim (16) split across 8 cores -> 2 batches (8 (b,c) slabs
of 1024x1024) per core.  Slabs stream HBM->SBUF as [128, 4096] half-slab
f32 tiles, both tensors issued from the SP HWDGE ring (issuing tgt from
the ACT sequencer stalls ACT's compute behind each dma_start's in-order
buffer-free wait, a measured ~10us standing backlog), 5-deep buffering.  The 16 SDMA engines then run ~100% busy at ~26.5 GB/s each --
~97% of the 435 GB/s SBUF-AXI fabric ceiling -- so the ~158us stream is a
hard floor and everything else must hide behind or hug its edges.
(Measured dead ends: bf16 cast-during-DMA via SWDGE runs ~27% slower per
engine -- the M2S/read side still moves f32 and the cast path adds
overhead; descriptor size 4/8/16 KB leaves per-engine rate unchanged.)

Compute is split across engines so neither lags the stream: the DVE does
the inter reduction per unit (fused scalar_tensor_tensor mult+mult with
accum_out) and the ACT engine does the two plain sums (activation Copy
with accum_out); per half-tile pair that is DVE 4.4us + ACT 7.8us against
11.4us of DMA.  The last slab is split into three quarters + an eighth +
two sixteenths, with the sums of two tail units moved to the DVE as fused
union ops (mult+add -> sum(pt+tt)) so each engine carries ~12us against
the ~19us tail DMA window and the post-stream drain is one sixteenth's
compute (descriptor cost stays linear down to 2KB, so the small tail
transfers do not slow the stream).

Per-partition partials land in a zero-initialised stats tile
[A(inter|psum|tsum) | B(...)]; after the last accumulation the whole
[128, 60] tile is DMA'd out raw (30KB/core).  The partition collapse, all
folds, and the dice arithmetic run on the host in fp64 while gathering
the 8 per-core tiles -- the device-side tail is just one DMA after the
final accumulator write.
"""

import numpy as np

import concourse.bass as bass
import concourse.tile as tile
from concourse import bacc, mybir
from concourse.bass_utils import run_bass_kernel_spmd

B, C, H, W = 16, 4, 1024, 1024
N_CORES = 8
P = 128
EPS = 1e-6

B_LOC = B // N_CORES          # batches per core
SLABS = B_LOC * C             # (b,c) slabs per core
F = (H * W) // P              # free dim per full slab

N_FOLD = SLABS + 2            # columns per (group, kind)
N_OUT = 6 * N_FOLD            # raw stats columns DMA'd out per core


def _build_nc(slabs: int, feat: int, n_cores: int):
    """Build + compile the per-core Bass program (same program on all cores)."""
    nc = bacc.Bacc(
        "TRN2", target_bir_lowering=False, debug=False, num_devices=n_cores
    )
    f32 = mybir.dt.float32
    half = feat // 2
    quarter = feat // 4
    eighth = feat // 8
    n_fold = slabs + 2
    prd = nc.dram_tensor("prd", [slabs, P, feat], f32, kind="ExternalInput")
    tgt = nc.dram_tensor("tgt", [slabs, P, feat], f32, kind="ExternalInput")
    out = nc.dram_tensor("out", [P, 6 * n_fold], f32, kind="ExternalOutput")

    add = mybir.AluOpType.add
    mult = mybir.AluOpType.mult
    copy_f = mybir.ActivationFunctionType.Copy

    # (slab, col_offset, width, fold_group, fold_idx, sums_on) units.
    # Full slabs in halves with the sums on ACT; the last slab as three
    # quarters + an eighth + two sixteenths, two of them with a fused DVE
    # union (sum(pt+tt) via scalar_tensor_tensor mult+add) to balance the
    # tail.
    ls = slabs - 1
    units = []
    for s in range(ls):
        units.append((s, 0, half, 0, s, "act"))
        units.append((s, half, half, 1, s, "act"))
    sixt = feat // 16
    units.append((ls, 0, quarter, 0, ls, "dve"))
    units.append((ls, quarter, quarter, 1, ls, "act"))
    units.append((ls, 2 * quarter, quarter, 0, ls + 1, "act"))
    units.append((ls, 3 * quarter, eighth, 1, ls + 1, "act"))
    units.append((ls, 3 * quarter + eighth, sixt, 0, ls + 2, "dve"))
    units.append((ls, 3 * quarter + eighth + sixt, sixt, 1, ls + 2, "act"))

    # stats column layout: [A | B], each group = [inter | psum | tsum]
    # (DVE-union units put sum(pt)+sum(tt) in the psum column, tsum stays 0)
    K_INTER, K_PSUM, K_TSUM = 0, 1, 2

    def col(g, kind, i):
        return 3 * n_fold * g + kind * n_fold + i

    with tile.TileContext(nc) as tc:
        with (
            tc.tile_pool(name="io", bufs=5) as io_pool,
            tc.tile_pool(name="work", bufs=1) as work_pool,
        ):
            stats = work_pool.tile([P, 6 * n_fold], f32)
            nc.vector.memset(stats[:], 0.0)
            scr_v = work_pool.tile([P, half], f32)   # DVE main-out sink
            scr_a = work_pool.tile([P, half], f32)   # ACT main-out sink

            # The SP ring generates descriptors sequentially (~57/us), so
            # with every transfer on one ring the slowest SDMA engine sees
            # its first descriptor ~4.5us after the first -- a 1:1 hit on
            # the stream end.  Hoist the first few tgt transfers onto the
            # otherwise-idle ACT ring, issued before any ACT compute (all
            # buffers are free, so the issues don't block): both rings
            # generate in parallel and every engine starts ~2.5us sooner.
            N_HOIST = 3
            hoisted = []
            for s, off, width, g, i, sums_on in units[:N_HOIST]:
                tt = io_pool.tile([P, width], f32, tag="tgt")
                nc.scalar.dma_start(tt[:], tgt[s, :, off : off + width])
                hoisted.append(tt)

            for u, (s, off, width, g, i, sums_on) in enumerate(units):
                # steady-state loads all issue from the SP sequencer: issuing
                # tgt from the ACT sequencer mid-stream stalls ACT's compute
                # behind the in-order buffer-free wait of each dma_start
                # (~10us standing backlog)
                pt = io_pool.tile([P, width], f32, tag="prd")
                nc.sync.dma_start(pt[:], prd[s, :, off : off + width])
                if u < N_HOIST:
                    tt = hoisted[u]
                else:
                    tt = io_pool.tile([P, width], f32, tag="tgt")
                    nc.sync.dma_start(tt[:], tgt[s, :, off : off + width])

                # inter partial on the DVE: accum_out = sum((pt*1) * tt)
                nc.vector.scalar_tensor_tensor(
                    out=scr_v[:, 0:width], in0=pt[:], scalar=1.0, in1=tt[:],
                    op0=mult, op1=mult,
                    accum_out=stats[:, col(g, K_INTER, i) : col(g, K_INTER, i) + 1],
                )
                if sums_on == "dve":
                    # fused union on the DVE: accum_out = sum((pt*1) + tt)
                    nc.vector.scalar_tensor_tensor(
                        out=scr_v[:, 0:width], in0=pt[:], scalar=1.0, in1=tt[:],
                        op0=mult, op1=add,
                        accum_out=stats[:, col(g, K_PSUM, i) : col(g, K_PSUM, i) + 1],
                    )
                else:
                    # plain sums on the ACT engine (accumulating Copy)
                    nc.scalar.activation(
                        out=scr_a[:, 0:width], in_=pt[:], func=copy_f,
                        accum_out=stats[:, col(g, K_PSUM, i) : col(g, K_PSUM, i) + 1],
                    )
                    nc.scalar.activation(
                        out=scr_a[:, 0:width], in_=tt[:], func=copy_f,
                        accum_out=stats[:, col(g, K_TSUM, i) : col(g, K_TSUM, i) + 1],
                    )

            # Ship the whole [P, 60] stats tile raw; the host collapses the
            # partition dim too (drops the matmul + PSUM-bounce semaphore
            # hops from the critical tail).
            nc.sync.dma_start(out[:, :], stats[:])

    nc.compile()
    return nc


def finalize(core_outs, slabs=SLABS, c=C, b=B):
    """Host-side fp64 reduction of the raw per-core stats vectors."""
    n_fold = slabs + 2
    total = np.zeros(c, dtype=np.float64)
    for o in core_outs:
        arr = np.asarray(o, dtype=np.float64).reshape(-1, 2, 3, n_fold)
        kinds = arr.sum(axis=(0, 1))               # fold partitions + groups
        # fold the tail columns (ls+1, ls+2) into the last-slab column
        per_slab = kinds[:, :slabs].copy()
        per_slab[:, slabs - 1] += kinds[:, slabs:].sum(axis=1)
        inter, psum, tsum = per_slab
        dice = (2.0 * inter + EPS) / (psum + tsum + EPS)   # (slabs,)
        total += dice.reshape(-1, c).sum(axis=0)           # fold local batches
    return (total / b).astype(np.float32)


_NC_CACHE: dict = {}


def _get_nc():
    key = (SLABS, F, N_CORES)
    if key not in _NC_CACHE:
        _NC_CACHE[key] = _build_nc(*key)
    return _NC_CACHE[key]


def _shard_inputs(prd: np.ndarray, tgt: np.ndarray):
    in_maps = []
    for i in range(N_CORES):
        sl = slice(i * B_LOC, (i + 1) * B_LOC)
        in_maps.append(
            {
                "prd": np.ascontiguousarray(prd[sl]).reshape(SLABS, P, F),
                "tgt": np.ascontiguousarray(tgt[sl]).reshape(SLABS, P, F),
            }
        )
    return in_maps


def kernel(prd: np.ndarray, tgt: np.ndarray, _trace: bool = False):
    prd = np.asarray(prd, dtype=np.float32)
    tgt = np.asarray(tgt, dtype=np.float32)
    assert prd.shape == (B, C, H, W) and tgt.shape == (B, C, H, W)

    nc = _get_nc()
    in_maps = _shard_inputs(prd, tgt)
    res = run_bass_kernel_spmd(nc, in_maps, list(range(N_CORES)), trace=_trace)
    out = finalize([r["out"] for r in res.results])
    if _trace:
        return out, res
    return out


# revision 31
# speedup vs baseline: 1.2166x; 1.2166x over previous
"""Dice coefficient metric kernel for TRN2 (8 NeuronCores, SPMD batch-parallel).

Reference computation (all fp32):
    inter[b,c] = sum_hw prd*tgt
    union[b,c] = sum_hw prd + sum_hw tgt + EPS
    dice[b,c]  = (2*inter + EPS) / union
    out[c]     = mean_b dice[b,c]

Sharding: batch dim (16) split across 8 cores -> 2 batches (8 (b,c) slabs
of 1024x1024) per core.  Slabs stream HBM->SBUF as [128, 4096] half-slab
f32 tiles, both tensors issued from the SP HWDGE ring (issuing tgt from
the ACT sequencer stalls ACT's compute behind each dma_start's in-order
buffer-free wait, a measured ~10us standing backlog), 5-deep buffering.  The 16 SDMA engines then run ~100% busy at ~26.5 GB/s each --
~97% of the 435 GB/s SBUF-AXI fabric ceiling -- so the ~158us stream is a
hard floor and everything else must hide behind or hug its edges.
(Measured dead ends: bf16 cast-during-DMA via SWDGE runs ~27% slower per
engine -- the M2S/read side still moves f32 and the cast path adds
overhead; descriptor size 4/8/16 KB leaves per-engine rate unchanged.)

Compute is split across engines so neither lags the stream: the DVE does
the inter reduction per unit (fused scalar_tensor_tensor mult+mult with
accum_out) and the ACT engine does the two plain sums (activation Copy
with accum_out); per half-tile pair that is DVE 4.4us + ACT 7.8us against
11.4us of DMA.  The last slab is split into three quarters + an eighth +
two sixteenths, with the sums of two tail units moved to the DVE as fused
union ops (mult+add -> sum(pt+tt)) so each engine carries ~12us against
the ~19us tail DMA window and the post-stream drain is one sixteenth's
compute (descriptor cost stays linear down to 2KB, so the small tail
transfers do not slow the stream).

Per-partition partials land in a zero-initialised stats tile
[A(inter|psum|tsum) | B(...)]; after the last accumulation the whole
[128, 60] tile is DMA'd out raw (30KB/core).  The partition collapse, all
folds, and the dice arithmetic run on the host in fp64 while gathering
the 8 per-core tiles -- the device-side tail is just one DMA after the
final accumulator write.
"""

import numpy as np

import concourse.bass as bass
import concourse.tile as tile
from concourse import bacc, mybir
from concourse.bass_utils import run_bass_kernel_spmd

B, C, H, W = 16, 4, 1024, 1024
N_CORES = 8
P = 128
EPS = 1e-6

B_LOC = B // N_CORES          # batches per core
SLABS = B_LOC * C             # (b,c) slabs per core
F = (H * W) // P              # free dim per full slab

N_FOLD = SLABS + 2            # columns per (group, kind)
N_OUT = 6 * N_FOLD            # raw stats columns DMA'd out per core


def _build_nc(slabs: int, feat: int, n_cores: int):
    """Build + compile the per-core Bass program (same program on all cores)."""
    nc = bacc.Bacc(
        "TRN2", target_bir_lowering=False, debug=False, num_devices=n_cores
    )
    f32 = mybir.dt.float32
    half = feat // 2
    quarter = feat // 4
    eighth = feat // 8
    n_fold = slabs + 2
    prd = nc.dram_tensor("prd", [slabs, P, feat], f32, kind="ExternalInput")
    tgt = nc.dram_tensor("tgt", [slabs, P, feat], f32, kind="ExternalInput")
    out = nc.dram_tensor("out", [P, 6 * n_fold], f32, kind="ExternalOutput")

    add = mybir.AluOpType.add
    mult = mybir.AluOpType.mult
    copy_f = mybir.ActivationFunctionType.Copy

    # (slab, col_offset, width, fold_group, fold_idx, sums_on) units.
    # Full slabs in halves with the sums on ACT; the last slab as three
    # quarters + an eighth + two sixteenths, two of them with a fused DVE
    # union (sum(pt+tt) via scalar_tensor_tensor mult+add) to balance the
    # tail.
    ls = slabs - 1
    units = []
    for s in range(ls):
        units.append((s, 0, half, 0, s, "act"))
        units.append((s, half, half, 1, s, "act"))
    sixt = feat // 16
    units.append((ls, 0, quarter, 0, ls, "dve"))
    units.append((ls, quarter, quarter, 1, ls, "act"))
    units.append((ls, 2 * quarter, quarter, 0, ls + 1, "act"))
    units.append((ls, 3 * quarter, eighth, 1, ls + 1, "act"))
    units.append((ls, 3 * quarter + eighth, sixt, 0, ls + 2, "act"))
    units.append((ls, 3 * quarter + eighth + sixt, sixt, 1, ls + 2, "dve"))

    # stats column layout: [A | B], each group = [inter | psum | tsum]
    # (DVE-union units put sum(pt)+sum(tt) in the psum column, tsum stays 0)
    K_INTER, K_PSUM, K_TSUM = 0, 1, 2

    def col(g, kind, i):
        return 3 * n_fold * g + kind * n_fold + i

    with tile.TileContext(nc) as tc:
        with (
            tc.tile_pool(name="io", bufs=5) as io_pool,
            tc.tile_pool(name="work", bufs=1) as work_pool,
        ):
            stats = work_pool.tile([P, 6 * n_fold], f32)
            nc.vector.memset(stats[:], 0.0)
            scr_v = work_pool.tile([P, half], f32)   # DVE main-out sink
            scr_a = work_pool.tile([P, half], f32)   # ACT main-out sink

            # The SP ring generates descriptors sequentially (~57/us), so
            # with every transfer on one ring the slowest SDMA engine sees
            # its first descriptor ~4.5us after the first -- a 1:1 hit on
            # the stream end.  Hoist the first few tgt transfers onto the
            # otherwise-idle ACT ring, issued before any ACT compute (all
            # buffers are free, so the issues don't block): both rings
            # generate in parallel and every engine starts ~2.5us sooner.
            N_HOIST = 3
            hoisted = []
            for s, off, width, g, i, sums_on in units[:N_HOIST]:
                tt = io_pool.tile([P, width], f32, tag="tgt")
                nc.scalar.dma_start(tt[:], tgt[s, :, off : off + width])
                hoisted.append(tt)

            for u, (s, off, width, g, i, sums_on) in enumerate(units):
                # steady-state loads all issue from the SP sequencer: issuing
                # tgt from the ACT sequencer mid-stream stalls ACT's compute
                # behind the in-order buffer-free wait of each dma_start
                # (~10us standing backlog)
                pt = io_pool.tile([P, width], f32, tag="prd")
                nc.sync.dma_start(pt[:], prd[s, :, off : off + width])
                if u < N_HOIST:
                    tt = hoisted[u]
                else:
                    tt = io_pool.tile([P, width], f32, tag="tgt")
                    nc.sync.dma_start(tt[:], tgt[s, :, off : off + width])

                # inter partial on the DVE: accum_out = sum((pt*1) * tt)
                nc.vector.scalar_tensor_tensor(
                    out=scr_v[:, 0:width], in0=pt[:], scalar=1.0, in1=tt[:],
                    op0=mult, op1=mult,
                    accum_out=stats[:, col(g, K_INTER, i) : col(g, K_INTER, i) + 1],
                )
                if sums_on == "dve":
                    # fused union on the DVE: accum_out = sum((pt*1) + tt)
                    nc.vector.scalar_tensor_tensor(
                        out=scr_v[:, 0:width], in0=pt[:], scalar=1.0, in1=tt[:],
                        op0=mult, op1=add,
                        accum_out=stats[:, col(g, K_PSUM, i) : col(g, K_PSUM, i) + 1],
                    )
                else:
                    # plain sums on the ACT engine (accumulating Copy)
                    nc.scalar.activation(
                        out=scr_a[:, 0:width], in_=pt[:], func=copy_f,
                        accum_out=stats[:, col(g, K_PSUM, i) : col(g, K_PSUM, i) + 1],
                    )
                    nc.scalar.activation(
                        out=scr_a[:, 0:width], in_=tt[:], func=copy_f,
                        accum_out=stats[:, col(g, K_TSUM, i) : col(g, K_TSUM, i) + 1],
                    )

            # Ship the whole [P, 60] stats tile raw; the host collapses the
            # partition dim too (drops the matmul + PSUM-bounce semaphore
            # hops from the critical tail).
            nc.sync.dma_start(out[:, :], stats[:])

    nc.compile()
    return nc


def finalize(core_outs, slabs=SLABS, c=C, b=B):
    """Host-side fp64 reduction of the raw per-core stats vectors."""
    n_fold = slabs + 2
    total = np.zeros(c, dtype=np.float64)
    for o in core_outs:
        arr = np.asarray(o, dtype=np.float64).reshape(-1, 2, 3, n_fold)
        kinds = arr.sum(axis=(0, 1))               # fold partitions + groups
        # fold the tail columns (ls+1, ls+2) into the last-slab column
        per_slab = kinds[:, :slabs].copy()
        per_slab[:, slabs - 1] += kinds[:, slabs:].sum(axis=1)
        inter, psum, tsum = per_slab
        dice = (2.0 * inter + EPS) / (psum + tsum + EPS)   # (slabs,)
        total += dice.reshape(-1, c).sum(axis=0)           # fold local batches
    return (total / b).astype(np.float32)


_NC_CACHE: dict = {}


def _get_nc():
    key = (SLABS, F, N_CORES)
    if key not in _NC_CACHE:
        _NC_CACHE[key] = _build_nc(*key)
    return _NC_CACHE[key]


def _shard_inputs(prd: np.ndarray, tgt: np.ndarray):
    in_maps = []
    for i in range(N_CORES):
        sl = slice(i * B_LOC, (i + 1) * B_LOC)
        in_maps.append(
            {
                "prd": np.ascontiguousarray(prd[sl]).reshape(SLABS, P, F),
                "tgt": np.ascontiguousarray(tgt[sl]).reshape(SLABS, P, F),
            }
        )
    return in_maps


def kernel(prd: np.ndarray, tgt: np.ndarray, _trace: bool = False):
    prd = np.asarray(prd, dtype=np.float32)
    tgt = np.asarray(tgt, dtype=np.float32)
    assert prd.shape == (B, C, H, W) and tgt.shape == (B, C, H, W)

    nc = _get_nc()
    in_maps = _shard_inputs(prd, tgt)
    res = run_bass_kernel_spmd(nc, in_maps, list(range(N_CORES)), trace=_trace)
    out = finalize([r["out"] for r in res.results])
    if _trace:
        return out, res
    return out
